# revision 4
# baseline (speedup 1.0000x reference)
"""Chamfer loss kernel v2 for Trainium2 (8 NeuronCores).

Augmented K=20 bf16 matmul (exact hi/lo split), per-core 4096x8192 slab,
ScalarE PSUM->SBUF fp16 extraction. Measured-tier-driven structure:
  * row-min via pairwise tensor_tensor min-TREE (fp16 2x mode) instead of
    tensor_scalar+accum_out (any op with an accumulator destination runs
    at 1x = 1 elem/cycle; tensor_tensor_reduce hard-crashes TRN2).
  * tree stops at FD=256 per row tile into a persistent buffer; one
    deferred [128,32,256]->[128,32] reduce after the loop.
  * col-min as one FD=8192 tensor_tensor per row tile (fewer DVE drains).
Measured ~295us HW exec vs 416-434us baseline (DVE-bound; ACT ~250us).
"""

import numpy as np

_NC_CACHE = None

_B = 4
_N = 8192
_H = 4096
_NCORES = 8
_NI = _H // 128
_GRP = 2048
_NG = _N // _GRP
_MM_N = 512
_K = 20

_ROW_MODE = "rtree"  # rtree | rttr | vmax


def _build_nc(compile_module=True, loop_repeats=None, row_mode=None):
    import concourse.bacc as bacc
    import concourse.mybir as mybir
    from concourse import masks
    from concourse.tile import TileContext

    row_mode = row_mode or _ROW_MODE
    f32 = mybir.dt.float32
    f16 = mybir.dt.float16
    bf16 = mybir.dt.bfloat16
    Alu = mybir.AluOpType

    nc = bacc.Bacc()
    uv = nc.dram_tensor("uv", [_K, _H + _N], bf16, kind="ExternalInput")
    out_x = nc.dram_tensor("out_x", [128, _NI], f32, kind="ExternalOutput")
    out_y = nc.dram_tensor("out_y", [128, _N // 128], f32, kind="ExternalOutput")

    with TileContext(nc) as tc:
        with (
            tc.tile_pool(name="const", bufs=1) as cpool,
            tc.tile_pool(name="work", bufs=3) as wpool,
            tc.tile_pool(name="psum", bufs=2, space="PSUM") as ppool,
        ):
            uv_sb = cpool.tile([32 + _K, _H + _N], bf16)
            nc.sync.dma_start(uv_sb[:_K, :], uv[:])
            nc.sync.dma_start(uv_sb[32 : 32 + _K, :], uv[:])
            u_bands = (uv_sb[:_K, :_H], uv_sb[32 : 32 + _K, :_H])
            v_bands = (uv_sb[:_K, _H:], uv_sb[32 : 32 + _K, _H:])

            ident = cpool.tile([128, 128], f16)
            masks.make_identity(nc, ident[:])

            colacc = cpool.tile([128, _N], f16)
            nc.vector.memset(colacc[:], 65504.0)

            rowmin = cpool.tile([128, _NI], f32)
            colmin = cpool.tile([128, _N // 128], f32)
            scr = cpool.tile([128, _N], f16)
            scr2 = cpool.tile([128, _NI * 256], f16)

            def main_block(_iv=None):
                mm_idx = 0
                for i in range(_NI):
                    s = wpool.tile([128, _N], f16, tag="s", name="s")
                    for g in range(_NG):
                        ps = ppool.tile([128, _GRP], f32, tag="mm", name="ps")
                        for k in range(_GRP // _MM_N):
                            c0 = g * _GRP + k * _MM_N
                            band = mm_idx % 2
                            mm_idx += 1
                            nc.tensor.matmul(
                                ps[:, k * _MM_N : (k + 1) * _MM_N],
                                u_bands[band][:, i * 128 : (i + 1) * 128],
                                v_bands[band][:, c0 : c0 + _MM_N],
                                start=True,
                                stop=True,
                                tile_position=(32 * band, 0),
                            )
                        nc.scalar.copy(s[:, g * _GRP : (g + 1) * _GRP], ps[:])
                    # col-min accumulate: one FD=8192 op (fp16 2x mode)
                    nc.vector.tensor_tensor(
                        colacc[:], s[:], colacc[:], Alu.min
                    )
                    # row-min
                    if row_mode == "rttr":
                        nc.vector.tensor_tensor_reduce(
                            scr[:, 0:4096],
                            s[:, 0:4096],
                            s[:, 4096:8192],
                            1.0,
                            65504.0,
                            Alu.min,
                            Alu.min,
                            accum_out=rowmin[:, i : i + 1],
                        )
                    else:  # rtree, stop at FD=256; final reduce deferred
                        nc.vector.tensor_tensor(
                            scr[:, 0:4096], s[:, 0:4096], s[:, 4096:8192], Alu.min
                        )
                        off, w = 0, 4096
                        while w > 512:
                            h = w // 2
                            nc.vector.tensor_tensor(
                                scr[:, off + w : off + w + h],
                                scr[:, off : off + h],
                                scr[:, off + h : off + w],
                                Alu.min,
                            )
                            off, w = off + w, h
                        # last level writes straight into the deferred buffer
                        nc.vector.tensor_tensor(
                            scr2[:, i * 256 : (i + 1) * 256],
                            scr[:, off : off + 256],
                            scr[:, off + 256 : off + 512],
                            Alu.min,
                        )

            if loop_repeats is None:
                main_block()
            else:
                with tc.For_i(0, loop_repeats, 1) as iv:
                    main_block(iv)

            nc.vector.tensor_reduce(
                rowmin[:],
                scr2.rearrange("p (a b) -> p a b", b=256),
                axis=mybir.AxisListType.X,
                op=Alu.min,
            )

            nblk = _N // 128
            for t in range(nblk // 4):
                tp = ppool.tile([128, 512], f16, tag="mm", name="tp")
                for k in range(4):
                    blk = t * 4 + k
                    nc.tensor.transpose(
                        tp[:, k * 128 : (k + 1) * 128],
                        colacc[:, blk * 128 : (blk + 1) * 128],
                        ident[:],
                    )
                nc.vector.tensor_reduce(
                    colmin[:, t * 4 : (t + 1) * 4],
                    tp.rearrange("p (b c) -> p b c", b=4),
                    axis=mybir.AxisListType.X,
                    op=Alu.min,
                )

            nc.sync.dma_start(out_x[:], rowmin[:])
            nc.sync.dma_start(out_y[:], colmin[:])
    if compile_module:
        nc.finalize()
    return nc


def _get_nc():
    global _NC_CACHE
    if _NC_CACHE is None:
        _NC_CACHE = _build_nc()
    return _NC_CACHE


def _hi_lo(a):
    import ml_dtypes

    hi = a.astype(ml_dtypes.bfloat16)
    lo = (a - hi.astype(np.float32)).astype(ml_dtypes.bfloat16)
    return hi, lo


def _make_in_maps(predictions, targets):
    import ml_dtypes

    bf16 = ml_dtypes.bfloat16
    in_maps = []
    for c in range(_NCORES):
        b, h = divmod(c, 2)
        x = np.asarray(predictions[b, h * _H : (h + 1) * _H], dtype=np.float32)
        y = np.asarray(targets[b], dtype=np.float32)
        u = np.empty((5, _H), np.float32)
        u[0:3] = x.T
        u[3] = (x * x).sum(axis=-1)
        u[4] = 1.0
        v = np.empty((5, _N), np.float32)
        v[0:3] = -2.0 * y.T
        v[3] = 1.0
        v[4] = (y * y).sum(axis=-1)
        u_hi, u_lo = _hi_lo(u)
        v_hi, v_lo = _hi_lo(v)
        uv = np.empty((_K, _H + _N), bf16)
        uv[0:5, :_H] = u_hi
        uv[5:10, :_H] = u_lo
        uv[10:15, :_H] = u_hi
        uv[15:20, :_H] = u_lo
        uv[0:5, _H:] = v_hi
        uv[5:10, _H:] = v_hi
        uv[10:15, _H:] = v_lo
        uv[15:20, _H:] = v_lo
        in_maps.append({"uv": uv})
    return in_maps


def _combine(results):
    loss = 0.0
    for b in range(_B):
        r0, r1 = results[2 * b], results[2 * b + 1]
        cx = np.concatenate(
            [
                np.ascontiguousarray(r0["out_x"].T).astype(np.float32).ravel(),
                np.ascontiguousarray(r1["out_x"].T).astype(np.float32).ravel(),
            ]
        )
        cy = np.minimum(
            np.ascontiguousarray(r0["out_y"].T).ravel(),
            np.ascontiguousarray(r1["out_y"].T).ravel(),
        )
        cx = np.maximum(cx, 0.0)
        cy = np.maximum(cy, 0.0)
        loss += cx.mean(dtype=np.float64) + cy.mean(dtype=np.float64)
    loss /= _B
    return np.array(loss, dtype=np.float32)


def kernel(predictions, targets):
    nc = _get_nc()
    in_maps = _make_in_maps(predictions, targets)
    try:
        from concourse.bass_utils import run_bass_kernel_spmd

        res = run_bass_kernel_spmd(nc, in_maps, core_ids=list(range(_NCORES)))
        results = res.results
    except ModuleNotFoundError:
        from concourse import bass2jax

        results = bass2jax.run_bass_via_pjrt(nc, in_maps, n_cores=_NCORES)
    return _combine(results)
